# revision 1
# baseline (speedup 1.0000x reference)
"""DJMGNN (NNConv/GraphNorm GNN) Trainium2 kernel, 8-core SPMD.

Sharding: nodes range-sharded N/8 per core; edges assigned to the core owning
their dst node, grouped into 128-node dst windows (window == node block).
Per layer:
  - h shard staged to DRAM and AllGathered into a global table
  - per-block indirect-DMA gather of h[src]
  - edge MLP on PE (attrT stationary, bias as ones-row) -> PSUM
  - fused relu*h on DVE (scalar_tensor_tensor from PSUM, o-major bcast AP)
  - one-hot scatter-matmul on PE accumulating aggI[u, (o,i)] per dst window
  - strided reduce over i on DVE
  - node path: PE transpose + root/transition matmuls (ones-row bias),
    GraphNorm stats via masked ones-column matmul, tiny AllReduce,
    reciprocal+sqrt+Newton for rstd, PE row-broadcast of scale/shift.
"""

import sys

if "/opt/trn_rl_repo" not in sys.path:
    sys.path.insert(0, "/opt/trn_rl_repo")

import numpy as np
import ml_dtypes

import concourse.bass as bass
import concourse.bacc as bacc
import concourse.mybir as mybir
import concourse.tile as tile

mdt = mybir.dt
AF = mybir.ActivationFunctionType
ALU = mybir.AluOpType

NCORES = 8
EPS = 1e-5


# ---------------------------------------------------------------- host prep


def _shard_graph(edge_index, n_nodes, shard, win=128, blk=128):
    src, dst = edge_index[0].astype(np.int64), edge_index[1].astype(np.int64)
    owner = dst // shard
    dst_local = dst - owner * shard
    nwin = (shard + win - 1) // win
    wid = dst_local // win

    lists = [[[] for _ in range(nwin)] for _ in range(NCORES)]
    for e in range(src.shape[0]):
        lists[owner[e]][wid[e]].append(e)

    bw = [
        max(max((len(lists[c][w]) + blk - 1) // blk, 1) for c in range(NCORES))
        for w in range(nwin)
    ]
    block_win = []
    for w in range(nwin):
        block_win += [w] * bw[w]
    nblocks = len(block_win)

    eslot = np.full((NCORES, nblocks, blk), -1, dtype=np.int64)
    for c in range(NCORES):
        b0 = 0
        for w in range(nwin):
            es = lists[c][w]
            for j, e in enumerate(es):
                eslot[c, b0 + j // blk, j % blk] = e
            b0 += bw[w]
    return eslot, block_win, bw, nwin


def prep_inputs(inputs):
    x = np.asarray(inputs["x"], np.float32)
    edge_attr = np.asarray(inputs["edge_attr"], np.float32)
    edge_index = np.asarray(inputs["edge_index"])
    N, IN = x.shape
    E, EA = edge_attr.shape
    H = np.asarray(inputs["init_W"]).shape[1]
    L = np.asarray(inputs["edge_mlp_W"]).shape[0]
    T = np.asarray(inputs["final_W"]).shape[1]
    shard = N // NCORES
    shard_pad = ((shard + 127) // 128) * 128
    nub = shard_pad // 128

    eslot, block_win, bw, nwin = _shard_graph(edge_index, N, shard)
    B = len(block_win)
    src = edge_index[0].astype(np.int64)
    dst = edge_index[1].astype(np.int64)

    # o-major reorder of edge MLP weights: col j = i*H + o -> o*H + i
    Wm = np.asarray(inputs["edge_mlp_W"], np.float32).reshape(L, EA, H, H)
    Wm = Wm.transpose(0, 1, 3, 2).reshape(L, EA, H * H)
    bm = np.asarray(inputs["edge_mlp_b"], np.float32).reshape(L, H, H)
    bm = bm.transpose(0, 2, 1).reshape(L, H * H)
    W_aug = np.concatenate([Wm, bm[:, None, :]], axis=1)  # [L, EA+1, H*H]

    init_aug = np.concatenate(
        [np.asarray(inputs["init_W"], np.float32),
         np.asarray(inputs["init_b"], np.float32)[None, :]], axis=0
    )  # [IN+1, H]

    rootW = np.asarray(inputs["root_W"], np.float32)
    root_aug = np.zeros((L, 2 * H + 1, H), np.float32)
    root_aug[:, :H, :] = rootW
    root_aug[:, 2 * H, :] = np.asarray(inputs["root_b"], np.float32)

    trans_aug = np.concatenate(
        [np.asarray(inputs["trans_W"], np.float32),
         np.asarray(inputs["trans_b"], np.float32)[:, None, :]], axis=1
    )  # [L, 2H+1, H]

    final_aug = np.zeros((2 * H + 1, T), np.float32)
    final_aug[:H, :] = np.asarray(inputs["final_W"], np.float32)
    final_aug[2 * H, :] = np.asarray(inputs["final_b"], np.float32)

    gn = np.concatenate(
        [np.asarray(inputs["gn_w"], np.float32),
         np.asarray(inputs["gn_b"], np.float32),
         np.asarray(inputs["gn_ms"], np.float32)], axis=1
    )[:, None, :]  # [L, 1, 3H]
    fgn = np.concatenate(
        [np.asarray(inputs["fgn_w"], np.float32),
         np.asarray(inputs["fgn_b"], np.float32),
         np.asarray(inputs["fgn_ms"], np.float32)], axis=0
    )[None, :]  # [1, 3T]

    iota = np.broadcast_to(np.arange(128, dtype=np.float32), (128, 128)).copy()
    ident = np.eye(128, dtype=np.float32)
    ones_row = np.ones((1, 128), np.float32)

    in_maps = []
    for c in range(NCORES):
        es = eslot[c]
        valid = es >= 0
        esc = np.where(valid, es, 0)

        attrT_aug = np.zeros((EA + 1, B * 128), np.float32)
        attrT_aug[:EA, :] = edge_attr[esc.reshape(-1)].T * valid.reshape(-1)
        attrT_aug[EA, :] = valid.reshape(-1).astype(np.float32)

        sg = src[esc.reshape(-1)]
        gidx = (sg // shard) * shard_pad + (sg % shard)
        gidx = np.where(valid.reshape(-1), gidx, 0).astype(np.int32)
        src_gidx = gidx.reshape(B, 128).T.copy()

        dl = dst[esc.reshape(-1)] - c * shard
        base = np.repeat(np.array([128 * w for w in block_win]), 128)
        dr = np.where(valid.reshape(-1), dl - base, -1.0).astype(np.float32)
        dst_rel = dr.reshape(B, 128).T.copy()

        xs = np.zeros((shard_pad, IN), np.float32)
        xs[:shard] = x[c * shard : (c + 1) * shard]
        xT_aug = np.concatenate([xs.T, np.ones((1, shard_pad), np.float32)], axis=0)
        xT_aug[IN, shard:] = 0.0

        mask = np.zeros((128, nub), np.float32)
        for u in range(nub):
            mask[: max(0, min(128, shard - u * 128)), u] = 1.0

        in_maps.append(
            {
                "attrT_aug": np.ascontiguousarray(attrT_aug).astype(ml_dtypes.bfloat16),
                "src_gidx": np.ascontiguousarray(src_gidx),
                "dst_rel": np.ascontiguousarray(dst_rel),
                "xT_aug": np.ascontiguousarray(xT_aug).astype(ml_dtypes.bfloat16),
                "mask": mask,
                "W_aug": W_aug.astype(ml_dtypes.bfloat16),
                "init_aug": init_aug.astype(ml_dtypes.bfloat16),
                "root_aug": root_aug.astype(ml_dtypes.bfloat16),
                "trans_aug": trans_aug.astype(ml_dtypes.bfloat16),
                "final_aug": final_aug.astype(ml_dtypes.bfloat16),
                "gn": gn,
                "fgn": fgn,
                "iota": iota,
                "ident": ident,
                "ones_row": ones_row,
            }
        )

    shapes = dict(
        N=N, E=E, IN=IN, H=H, EA=EA, T=T, L=L, shard=shard, shard_pad=shard_pad,
        nub=nub, B=B, block_win=tuple(block_win), bw=tuple(bw), nwin=nwin,
    )
    return in_maps, shapes


# ------------------------------------------------------------- device build


def build_program(s):
    H, EA, IN, T, L = s["H"], s["EA"], s["IN"], s["T"], s["L"]
    B, nub, nwin = s["B"], s["nub"], s["nwin"]
    shard_pad = s["shard_pad"]
    block_win = s["block_win"]
    HH = H * H
    HHH = HH // 2
    n_total = shard_pad * NCORES
    n_real = s["N"]

    nc = bacc.Bacc("TRN2", target_bir_lowering=False, debug=False,
                   enable_asserts=False, num_devices=NCORES)

    def din(name, shape, dtype=mdt.float32):
        return nc.dram_tensor(name, shape, dtype, kind="ExternalInput").ap()

    attrT = din("attrT_aug", [EA + 1, B * 128], mdt.bfloat16)
    src_gidx = din("src_gidx", [128, B], mdt.int32)
    dst_rel = din("dst_rel", [128, B])
    xT_aug = din("xT_aug", [IN + 1, shard_pad], mdt.bfloat16)
    mask_in = din("mask", [128, nub])
    W_in = din("W_aug", [L, EA + 1, HH], mdt.bfloat16)
    init_in = din("init_aug", [IN + 1, H], mdt.bfloat16)
    root_in = din("root_aug", [L, 2 * H + 1, H], mdt.bfloat16)
    trans_in = din("trans_aug", [L, 2 * H + 1, H], mdt.bfloat16)
    final_in = din("final_aug", [2 * H + 1, T], mdt.bfloat16)
    gn_in = din("gn", [L, 1, 3 * H])
    fgn_in = din("fgn", [1, 3 * T])
    iota_in = din("iota", [128, 128])
    ident_in = din("ident", [128, 128])
    ones_in = din("ones_row", [1, 128])

    out_dram = nc.dram_tensor("out", [shard_pad, T], mdt.float32,
                              kind="ExternalOutput").ap()

    rg = [list(range(NCORES))]

    with tile.TileContext(nc) as tc:
        with (
            tc.tile_pool(name="const", bufs=1) as cpool,
            tc.tile_pool(name="hbuf", bufs=1) as hpool,
            tc.tile_pool(name="tmp", bufs=8) as tmppool,
            tc.tile_pool(name="rows", bufs=10) as rpool,
            tc.tile_pool(name="ps", bufs=4, space="PSUM") as ps,
            tc.tile_pool(name="dram", bufs=1, space="DRAM") as dram,
        ):
            def load(pool, shape, ap, dtype=mdt.float32, tag=None):
                t = pool.tile(shape, dtype, tag=tag)
                nc.sync.dma_start(t[:], ap)
                return t

            attrT_sb = load(cpool, [EA + 1, B * 128], attrT[:], mdt.bfloat16, tag="attrT")
            dst_sb = load(cpool, [128, B], dst_rel[:], tag="dstrel")
            idx_sb = load(cpool, [128, B], src_gidx[:], mdt.int32, tag="sidx")
            xT_sb = load(cpool, [IN + 1, shard_pad], xT_aug[:], mdt.bfloat16, tag="xT")
            mask_sb = load(cpool, [128, nub], mask_in[:], tag="mask")
            init_sb = load(cpool, [IN + 1, H], init_in[:], mdt.bfloat16, tag="initw")
            final_sb = load(cpool, [2 * H + 1, T], final_in[:], mdt.bfloat16, tag="finalw")
            iota_sb = load(cpool, [128, 128], iota_in[:], tag="iota")
            ident_sb = load(cpool, [128, 128], ident_in[:], tag="ident")
            onesr_sb = load(cpool, [1, 128], ones_in[:], tag="onesr")
            fgn_sb = load(cpool, [1, 3 * T], fgn_in[:], tag="fgn")
            W_l = [load(cpool, [EA + 1, HH], W_in[li], mdt.bfloat16, tag=f"W{li}")
                   for li in range(L)]
            root_l = [load(cpool, [2 * H + 1, H], root_in[li], mdt.bfloat16,
                           tag=f"rw{li}") for li in range(L)]
            trans_l = [load(cpool, [2 * H + 1, H], trans_in[li], mdt.bfloat16,
                            tag=f"tw{li}") for li in range(L)]
            gn_l = [load(cpool, [1, 3 * H], gn_in[li], tag=f"gn{li}")
                    for li in range(L)]

            # ---- one-hot blocks (graph-constant, bf16)
            onehot_sb = cpool.tile([128, B, 128], mdt.bfloat16)
            for b in range(B):
                nc.vector.tensor_scalar(
                    onehot_sb[:, b, :], iota_sb[:], dst_sb[:, b : b + 1],
                    None, op0=ALU.is_equal,
                )

            # ---- persistent tiles
            hA = hpool.tile([128, nub, H], mdt.float32)
            hB = hpool.tile([128, nub, H], mdt.float32)
            agg_sb = hpool.tile([128, nwin, H], mdt.float32)
            conv_sb = hpool.tile([128, nub, H], mdt.float32)
            hsrc_sb = hpool.tile([128, B, H], mdt.bfloat16)
            hstage_sb = hpool.tile([128, nub, H], mdt.bfloat16)
            cd_sb = hpool.tile([128, 2 * H], mdt.float32)
            fcd_sb = hpool.tile([128, 2 * T], mdt.float32)
            fo_sb = hpool.tile([128, nub, T], mdt.float32)
            catT_all = hpool.tile([2 * H + 1, nub, 128], mdt.bfloat16)
            stats_sb = hpool.tile([1, 2 * H + 2], mdt.float32)
            fstats_sb = hpool.tile([1, 2 * T + 2], mdt.float32)

            # init: hc rows zero (avoid NaN garbage x 0-weights), ones row
            nc.vector.memset(catT_all[H : 2 * H, :, :], 0.0)
            nc.vector.memset(catT_all[2 * H : 2 * H + 1, :, :], 1.0)

            hstage_dram = dram.tile([shard_pad, H], mdt.bfloat16)
            htable_l = [dram.tile([n_total, H], mdt.bfloat16, addr_space="Shared",
                                  tag=f"htable{li}", name=f"htable{li}") for li in range(L)]
            st_in = dram.tile([1, 2 * H + 2], mdt.float32)
            st_out_l = [dram.tile([1, 2 * H + 2], mdt.float32, addr_space="Shared",
                                  tag=f"stout{li}", name=f"stout{li}") for li in range(L)]
            fst_in = dram.tile([1, 2 * T + 2], mdt.float32)
            fst_out = dram.tile([1, 2 * T + 2], mdt.float32, addr_space="Shared")

            hstage_v = hstage_dram[:].rearrange("(u p) f -> p u f", p=128)

            def rstd_row(dstrow, varrow, width, tag):
                """dstrow = 1/sqrt(varrow+EPS) via reciprocal+sqrt+Newton."""
                ve = rpool.tile([1, width], mdt.float32, tag=tag)
                nc.vector.tensor_scalar_add(ve[:], varrow, EPS)
                r2 = rpool.tile([1, width], mdt.float32, tag=tag)
                nc.vector.reciprocal(r2[:], ve[:])
                r0 = rpool.tile([1, width], mdt.float32, tag=tag)
                nc.scalar.activation(r0[:], r2[:], AF.Sqrt)
                # Newton: r = r0*(1.5 - 0.5*ve*r0^2)
                t0 = rpool.tile([1, width], mdt.float32, tag=tag)
                nc.vector.tensor_mul(t0[:], r0[:], r0[:])
                nc.vector.tensor_mul(t0[:], t0[:], ve[:])
                nc.vector.scalar_tensor_tensor(
                    t0[:], t0[:], -0.5, r0[:], op0=ALU.mult, op1=ALU.mult
                )
                nc.vector.scalar_tensor_tensor(
                    dstrow, r0[:], 1.5, t0[:], op0=ALU.mult, op1=ALU.add
                )

            def cd_rows(crow, srow, gnw, gnb, gnms, width, tag):
                """crow[0:w] = C = rstd*w ; crow[w:2w] = D = b - ms*mean*C."""
                mean = rpool.tile([1, width], mdt.float32, tag=tag)
                nc.vector.tensor_scalar_mul(mean[:], srow[:, width : 2 * width],
                                            1.0 / n_real)
                msq = rpool.tile([1, width], mdt.float32, tag=tag)
                nc.vector.tensor_scalar_mul(msq[:], srow[:, 0:width], 1.0 / n_real)
                mm = rpool.tile([1, width], mdt.float32, tag=tag)
                nc.vector.tensor_mul(mm[:], mean[:], mean[:])
                nc.vector.tensor_mul(mm[:], mm[:], gnms)
                co = rpool.tile([1, width], mdt.float32, tag=tag)
                nc.vector.tensor_scalar(co[:], gnms, -1.0, 2.0, op0=ALU.mult,
                                        op1=ALU.add)
                nc.vector.tensor_mul(mm[:], mm[:], co[:])
                var = rpool.tile([1, width], mdt.float32, tag=tag)
                nc.vector.tensor_sub(var[:], msq[:], mm[:])
                rstd = rpool.tile([1, width], mdt.float32, tag=tag)
                rstd_row(rstd[:], var[:], width, tag)
                nc.vector.tensor_mul(crow[:, 0:width], rstd[:], gnw)
                nc.vector.tensor_mul(crow[:, width : 2 * width], mean[:], gnms)
                nc.vector.tensor_mul(crow[:, width : 2 * width],
                                     crow[:, width : 2 * width], crow[:, 0:width])
                nc.vector.scalar_tensor_tensor(
                    crow[:, width : 2 * width], crow[:, width : 2 * width],
                    -1.0, gnb, op0=ALU.mult, op1=ALU.add,
                )

            # ============ layer 0: h0 = x @ init_W + b ============
            for u in range(nub):
                p = ps.tile([128, H], mdt.float32, tag="big")
                nc.tensor.matmul(p[:], xT_sb[:, u * 128 : (u + 1) * 128],
                                 init_sb[:], start=True, stop=True)
                nc.scalar.activation(hA[:, u, :], p[:], AF.Copy)

            hcur, hnxt = hA, hB
            for li in range(L):
                # ---- stage h + AllGather + gather h[src]
                if li == 0:
                    nc.vector.tensor_copy(hstage_sb[:], hcur[:])
                nc.sync.dma_start(hstage_v, hstage_sb[:])
                htable_dram = htable_l[li]
                nc.gpsimd.collective_compute(
                    "AllGather", ALU.bypass, replica_groups=rg,
                    ins=[hstage_dram.opt()], outs=[htable_dram.opt()],
                )
                for b in range(B):
                    nc.gpsimd.indirect_dma_start(
                        out=hsrc_sb[:, b, :],
                        out_offset=None,
                        in_=htable_dram[:],
                        in_offset=bass.IndirectOffsetOnAxis(
                            ap=idx_sb[:, b : b + 1], axis=0
                        ),
                    )

                # ---- edge phase
                aggI = None
                for b in range(B):
                    w = block_win[b]
                    first = b == 0 or block_win[b - 1] != w
                    last = b == B - 1 or block_win[b + 1] != w
                    pre = ps.tile([128, HH], mdt.float32, tag="big")
                    a_sl = attrT_sb[:, b * 128 : (b + 1) * 128]
                    nc.tensor.matmul(pre[:, 0:HHH], a_sl, W_l[li][:, 0:HHH],
                                     start=True, stop=True)
                    nc.tensor.matmul(pre[:, HHH:HH], a_sl, W_l[li][:, HHH:HH],
                                     start=True, stop=True)
                    tmp = tmppool.tile([128, HH], mdt.bfloat16, tag="tmp")
                    if b % 4 == 3:
                        nc.vector.scalar_tensor_tensor(
                            tmp[:].rearrange("p (o i) -> p o i", o=H, i=H),
                            pre[:].rearrange("p (o i) -> p o i", o=H, i=H),
                            0.0,
                            hsrc_sb[:, b, :].unsqueeze(1)
                            .broadcast_to([128, H, H]),
                            op0=ALU.max, op1=ALU.mult,
                        )
                    else:
                        ew = tmppool.tile([128, HH], mdt.bfloat16, tag="ew")
                        nc.scalar.activation(ew[:], pre[:], AF.Relu)
                        nc.vector.tensor_tensor(
                            tmp[:].rearrange("p (o i) -> p o i", o=H, i=H),
                            ew[:].rearrange("p (o i) -> p o i", o=H, i=H),
                            hsrc_sb[:, b, :].unsqueeze(1)
                            .broadcast_to([128, H, H]),
                            op=ALU.mult,
                        )
                    if first:
                        aggI = ps.tile([128, HH], mdt.float32, tag="big")
                    nc.tensor.matmul(aggI[:, 0:HHH], onehot_sb[:, b, :],
                                     tmp[:, 0:HHH], start=first, stop=last)
                    nc.tensor.matmul(aggI[:, HHH:HH], onehot_sb[:, b, :],
                                     tmp[:, HHH:HH], start=first, stop=last)
                    if last:
                        nc.vector.tensor_reduce(
                            agg_sb[:, w, :],
                            aggI[:].rearrange("p (o i) -> p o i", o=H, i=H),
                            axis=mybir.AxisListType.X, op=ALU.add,
                        )

                # ---- node pass 1: conv + stats
                nc.vector.memset(stats_sb[:], 0.0)
                for u in range(nub):
                    hT_ps = ps.tile([H, 128], mdt.float32, tag="big")
                    nc.tensor.transpose(hT_ps[:], hcur[:, u, :], ident_sb[:])
                    nc.scalar.activation(catT_all[0:H, u, :], hT_ps[:], AF.Copy)
                    rt_ps = ps.tile([128, H], mdt.float32, tag="big")
                    nc.tensor.matmul(rt_ps[:], catT_all[:, u, :], root_l[li][:],
                                     start=True, stop=True)
                    nc.vector.tensor_add(conv_sb[:, u, :], agg_sb[:, u, :],
                                         rt_ps[:])
                    st_tile = tmppool.tile([128, 2 * H + 1], mdt.float32,
                                           tag="strow")
                    nc.vector.tensor_mul(st_tile[:, 0:H], conv_sb[:, u, :],
                                         conv_sb[:, u, :])
                    nc.vector.tensor_copy(st_tile[:, H : 2 * H],
                                          conv_sb[:, u, :])
                    nc.vector.tensor_copy(st_tile[:, 2 * H : 2 * H + 1],
                                          mask_sb[:, u : u + 1])
                    smm_ps = ps.tile([1, 2 * H + 1], mdt.float32, tag="big")
                    nc.tensor.matmul(smm_ps[:], mask_sb[:, u : u + 1],
                                     st_tile[:], start=True, stop=True)
                    nc.vector.tensor_add(stats_sb[:, 0 : 2 * H + 1],
                                         stats_sb[:, 0 : 2 * H + 1], smm_ps[:])

                # ---- stats AllReduce
                nc.sync.dma_start(st_in[:], stats_sb[:])
                st_out = st_out_l[li]
                nc.gpsimd.collective_compute(
                    "AllReduce", ALU.add, replica_groups=rg,
                    ins=[st_in.opt()], outs=[st_out.opt()],
                )
                srow2 = rpool.tile([1, 2 * H + 2], mdt.float32, tag="srow")
                nc.sync.dma_start(srow2[:], st_out[:])

                # ---- C/D rows + broadcast
                crow = rpool.tile([1, 2 * H], mdt.float32, tag="cdrow")
                cd_rows(crow, srow2, gn_l[li][:, 0:H], gn_l[li][:, H : 2 * H],
                        gn_l[li][:, 2 * H : 3 * H], H, "nrow")
                cd_ps = ps.tile([128, 2 * H], mdt.float32, tag="big")
                nc.tensor.matmul(cd_ps[:], onesr_sb[:], crow[:], start=True,
                                 stop=True)
                nc.scalar.activation(cd_sb[:], cd_ps[:], AF.Copy)

                # ---- node pass 2
                for u in range(nub):
                    hc = tmppool.tile([128, H], mdt.float32, tag="hc")
                    nc.vector.tensor_mul(hc[:], conv_sb[:, u, :], cd_sb[:, 0:H])
                    nc.vector.tensor_add(hc[:], hc[:], cd_sb[:, H : 2 * H])
                    nc.vector.tensor_scalar_max(hc[:], hc[:], 0.0)
                    nc.vector.tensor_add(hc[:], hc[:], hcur[:, u, :])
                    hcT_ps = ps.tile([H, 128], mdt.float32, tag="big")
                    nc.tensor.transpose(hcT_ps[:], hc[:], ident_sb[:])
                    nc.scalar.activation(catT_all[H : 2 * H, u, :], hcT_ps[:],
                                         AF.Copy)
                    tr_ps = ps.tile([128, H], mdt.float32, tag="big")
                    nc.tensor.matmul(tr_ps[:], catT_all[:, u, :],
                                     trans_l[li][:], start=True, stop=True)
                    nc.scalar.activation(hnxt[:, u, :], tr_ps[:], AF.Relu)
                    nc.vector.tensor_copy(hstage_sb[:, u, :], hnxt[:, u, :])

                hcur, hnxt = hnxt, hcur

            # ============ final ============
            nc.vector.memset(fstats_sb[:], 0.0)
            for u in range(nub):
                hT_ps = ps.tile([H, 128], mdt.float32, tag="big")
                nc.tensor.transpose(hT_ps[:], hcur[:, u, :], ident_sb[:])
                nc.scalar.activation(catT_all[0:H, u, :], hT_ps[:], AF.Copy)
                nc.vector.memset(catT_all[H : 2 * H, u, :], 0.0)
                f_ps = ps.tile([128, T], mdt.float32, tag="big")
                nc.tensor.matmul(f_ps[:], catT_all[:, u, :], final_sb[:],
                                 start=True, stop=True)
                nc.scalar.activation(fo_sb[:, u, :], f_ps[:], AF.Copy)
                st_tile = tmppool.tile([128, 2 * T + 1], mdt.float32,
                                       tag="fstrow")
                nc.vector.tensor_mul(st_tile[:, 0:T], fo_sb[:, u, :],
                                     fo_sb[:, u, :])
                nc.vector.tensor_copy(st_tile[:, T : 2 * T], fo_sb[:, u, :])
                nc.vector.tensor_copy(st_tile[:, 2 * T : 2 * T + 1],
                                      mask_sb[:, u : u + 1])
                smm_ps = ps.tile([1, 2 * T + 1], mdt.float32, tag="big")
                nc.tensor.matmul(smm_ps[:], mask_sb[:, u : u + 1], st_tile[:],
                                 start=True, stop=True)
                nc.vector.tensor_add(fstats_sb[:, 0 : 2 * T + 1],
                                     fstats_sb[:, 0 : 2 * T + 1], smm_ps[:])

            nc.sync.dma_start(fst_in[:], fstats_sb[:])
            nc.gpsimd.collective_compute(
                "AllReduce", ALU.add, replica_groups=rg,
                ins=[fst_in.opt()], outs=[fst_out.opt()],
            )
            fsrow2 = rpool.tile([1, 2 * T + 2], mdt.float32, tag="fsrow")
            nc.sync.dma_start(fsrow2[:], fst_out[:])

            fcrow = rpool.tile([1, 2 * T], mdt.float32, tag="fcdrow")
            cd_rows(fcrow, fsrow2, fgn_sb[:, 0:T], fgn_sb[:, T : 2 * T],
                    fgn_sb[:, 2 * T : 3 * T], T, "frow")
            fcd_ps = ps.tile([128, 2 * T], mdt.float32, tag="big")
            nc.tensor.matmul(fcd_ps[:], onesr_sb[:], fcrow[:], start=True,
                             stop=True)
            nc.scalar.activation(fcd_sb[:], fcd_ps[:], AF.Copy)

            out_v = out_dram.rearrange("(u p) f -> p u f", p=128)
            for u in range(nub):
                y = tmppool.tile([128, T], mdt.float32, tag="y")
                nc.vector.tensor_mul(y[:], fo_sb[:, u, :], fcd_sb[:, 0:T])
                nc.vector.tensor_add(y[:], y[:], fcd_sb[:, T : 2 * T])
                nc.vector.tensor_scalar_max(y[:], y[:], 0.0)
                nc.sync.dma_start(out_v[:, u, :], y[:])

    nc.compile()
    return nc


# ------------------------------------------------------------------ driver

_CACHE = {}


def kernel(**inputs) -> np.ndarray:
    in_maps, s = prep_inputs(inputs)
    key = (s["N"], s["E"], s["B"], s["block_win"])
    if key not in _CACHE:
        _CACHE[key] = build_program(s)
    nc = _CACHE[key]

    from concourse.bass_utils import run_bass_kernel_spmd

    res = run_bass_kernel_spmd(nc, in_maps, core_ids=list(range(NCORES)))
    shard = s["shard"]
    outs = [res.results[c]["out"][:shard] for c in range(NCORES)]
    return np.concatenate(outs, axis=0).astype(np.float32)

